# revision 1
# baseline (speedup 1.0000x reference)
"""GCLSTM cell (Chebyshev K=3 GCN-gated LSTM) on 8 Trainium2 NeuronCores.

Sharding: nodes are partitioned contiguously across the 8 cores (12500 each).
Each core owns its node rows of X/H/C and the edges *incoming* to its nodes
(partitioned by destination column). Host pre-normalizes edge weights
(sym Laplacian scaling, cached graph preprocessing) and sorts/pads each
device's edges by (destination tile, source chunk).

Device pipeline per core:
  prop1: Tx1_i = L_hat @ H   via bulk dma_gather of H rows (4 SWDGE queues)
         + one-hot scatter matmuls on TensorE (PSUM accumulation per col tile)
  AllGather Tx1 shards -> Tx1_full (on-chip collective)
  prop2: 2 * L_hat @ Tx1 (weights pre-doubled), produced transposed;
         Tx2_T = psum - H_T
  dense: G[node, 512] = X@Wx + H@Cw0 + Tx1@Cw1 + Tx2@Cw2 + bias (4 gates)
  LSTM pointwise: C' = sig(F)*C + sig(I)*tanh(Tc); H' = sig(O)*tanh(C')

diag term of L_hat is exactly 0 (lambda_max = 2), so prop is pure scatter.
"""
import numpy as np

N = 100000
D = 128
NCORES = 8
NPC = N // NCORES            # 12500 nodes per core
TILES = (NPC + 127) // 128   # 98
NPAD = TILES * 128           # 12544
# Quarter-block source chunking: each gather-source block is the concat over
# ranks of one quarter of each rank's shard (block <= 31744 rows, under the
# int16 dma_gather index limit). Quarter boundaries are tile(128)-aligned;
# the last quarter is small so its AllGather tail is cheap.
QB = [0, 3968, 7936, 11904, 12500]
QSZ = [QB[i + 1] - QB[i] for i in range(4)]      # 3968,3968,3968,596
NSC = 4

_CACHE = {}


def _host_prep(X, edge_index, edge_weight, H, C, W, b, conv_W, conv_b):
    row = np.asarray(edge_index[0], dtype=np.int64)
    col = np.asarray(edge_index[1], dtype=np.int64)
    ew = np.asarray(edge_weight, dtype=np.float32)

    deg = np.bincount(row, weights=ew.astype(np.float64), minlength=N)
    deg = deg.astype(np.float32)
    dinv = np.where(deg > 0, deg ** -0.5, 0.0).astype(np.float32)
    w = -(dinv[row] * ew * dinv[col])  # 2/lambda_max == 1

    dev = col // NPC
    per_dev = []
    # First pass: bucket counts per (device, tile, src_chunk)
    counts = np.zeros((NCORES, TILES, NSC), dtype=np.int64)
    lsrc = row % NPC
    q_of = np.minimum(lsrc // 3968, 3)
    order = np.argsort(dev * (TILES * NSC) +
                       ((col % NPC) // 128) * NSC + q_of,
                       kind="stable")
    row_s, col_s, w_s, dev_s = row[order], col[order], w[order], dev[order]
    colloc_s = col_s % NPC
    tile_s = colloc_s // 128
    lsrc_s = row_s % NPC
    dsrc_s = row_s // NPC
    sc_s = np.minimum(lsrc_s // 3968, 3)
    qb = np.array(QB[:4], dtype=np.int64)
    qsz = np.array(QSZ, dtype=np.int64)
    blockrow_s = dsrc_s * qsz[sc_s] + (lsrc_s - qb[sc_s])
    np.add.at(counts, (dev_s, tile_s, sc_s), 1)

    # Uniform chunk capacity per src chunk (same program on all cores)
    cap = np.zeros(NSC, dtype=np.int64)
    for s in range(NSC):
        cap[s] = int(np.ceil(counts[:, :, s].max() / 128))
    cap = np.maximum(cap, 1)
    ntot = int(cap.sum()) * 128          # padded edges per tile
    nchunks = int(cap.sum())             # 128-edge chunks per tile

    # slot base offset of (tile, src_chunk) within a device's padded edge list
    sc_base = np.concatenate([[0], np.cumsum(cap)[:-1]]) * 128

    idx16 = np.zeros((NCORES, TILES * ntot), dtype=np.int16)
    coloff = np.zeros((NCORES, TILES * ntot), dtype=np.float32)
    wpad = np.zeros((NCORES, TILES * ntot), dtype=np.float32)

    # position of each edge within its (dev, tile, sc) bucket
    key = dev_s * (TILES * NSC) + tile_s * NSC + sc_s
    # edges are sorted by key; rank within bucket:
    diff = np.empty(len(key), dtype=np.int64)
    diff[0] = 0
    same = key[1:] == key[:-1]
    runstart = np.zeros(len(key), dtype=np.int64)
    idxs = np.arange(len(key))
    starts = np.concatenate([[0], idxs[1:][~same]])
    runid = np.cumsum(np.concatenate([[0], (~same).astype(np.int64)]))
    rank = idxs - starts[runid]

    slot = tile_s * ntot + sc_base[sc_s] + rank
    flat_dev = dev_s
    idx16[flat_dev, slot] = blockrow_s.astype(np.int16)
    coloff[flat_dev, slot] = (colloc_s % 128).astype(np.float32)
    wpad[flat_dev, slot] = w_s

    # SBUF layouts:
    #  - dma_gather idx: idx i -> partition i%16, column i//16; replicate x8
    #  - per-chunk scalars (coloff, w): lane e -> partition e, column chunk
    ncols_idx = TILES * ntot // 16
    nchunk_tot = TILES * nchunks
    for d in range(NCORES):
        a = idx16[d].reshape(-1, 16).T            # [16, ncols_idx]
        ii = np.tile(a, (8, 1))                   # replicate to 128 partitions
        co = coloff[d].reshape(nchunk_tot, 128).T  # [128, nchunk_tot]
        ww = wpad[d].reshape(nchunk_tot, 128).T
        per_dev.append((ii, co, ww))

    # fused dense weights: rhs blocks [128f, 512gc] for X, H(Tx0), Tx1, Tx2
    Wb = np.zeros((4, D, 4 * D), dtype=np.float32)
    for g in range(4):
        Wb[0][:, g * D:(g + 1) * D] = W[g]
        Wb[1][:, g * D:(g + 1) * D] = conv_W[g, 0]
        Wb[2][:, g * D:(g + 1) * D] = conv_W[g, 1]
        Wb[3][:, g * D:(g + 1) * D] = conv_W[g, 2]
    bias = np.concatenate([b[g] + conv_b[g] for g in range(4)]).astype(np.float32)
    biasb = np.tile(bias[None, :], (128, 1))

    Xp = np.zeros((NCORES, NPAD, D), np.float32)
    Hp = np.zeros((NCORES, NPAD, D), np.float32)
    Cp = np.zeros((NCORES, NPAD, D), np.float32)
    Xs = np.asarray(X, np.float32).reshape(NCORES, NPC, D)
    Hs = np.asarray(H, np.float32).reshape(NCORES, NPC, D)
    Cs = np.asarray(C, np.float32).reshape(NCORES, NPC, D)
    Xp[:, :NPC] = Xs
    Hp[:, :NPC] = Hs
    Cp[:, :NPC] = Cs
    XT = np.ascontiguousarray(np.transpose(Xp, (0, 2, 1)))  # [NC, D, NPAD]
    HT = np.ascontiguousarray(np.transpose(Hp, (0, 2, 1)))

    Hsh = np.asarray(H, np.float32).reshape(NCORES, NPC, D)
    hc = []
    for q in range(NSC):
        hc.append(np.ascontiguousarray(
            Hsh[:, QB[q]:QB[q + 1], :].reshape(-1, D)))

    in_maps = []
    for d in range(NCORES):
        ii, co, ww = per_dev[d]
        m = {
            "XT": XT[d], "HT": HT[d], "Cp": Cp[d],
            "idx": np.ascontiguousarray(ii),
            "coloff": np.ascontiguousarray(co),
            "w1": np.ascontiguousarray(ww),
            "Wb": Wb.reshape(4 * D, 4 * D),
            "biasb": biasb,
        }
        for s in range(NSC):
            m[f"Hc{s}"] = hc[s]
        in_maps.append(m)

    meta = dict(cap=tuple(int(c) for c in cap), ntot=ntot, nchunks=nchunks,
                ncols_idx=ncols_idx, nchunk_tot=nchunk_tot)
    return in_maps, meta


def _build_program(meta, variant="full", reps=1):
    import concourse.bass as bass
    import concourse.bacc as bacc
    import concourse.tile as tile
    from concourse import mybir
    from concourse.masks import make_identity

    cap = meta["cap"]
    ntot = meta["ntot"]
    nchunks = meta["nchunks"]
    ncols_idx = meta["ncols_idx"]
    nchunk_tot = meta["nchunk_tot"]
    capmax = max(cap)
    f32 = mybir.dt.float32

    nc = bacc.Bacc("TRN2", target_bir_lowering=False, debug=False,
                   num_devices=NCORES, num_swdge_queues=4)

    Hc = [nc.dram_tensor(f"Hc{s}", [NCORES * QSZ[s], D], f32,
                         kind="ExternalInput") for s in range(NSC)]
    XTd = nc.dram_tensor("XT", [D, NPAD], f32, kind="ExternalInput")
    HTd = nc.dram_tensor("HT", [D, NPAD], f32, kind="ExternalInput")
    Cp = nc.dram_tensor("Cp", [NPAD, D], f32, kind="ExternalInput")
    IDX = nc.dram_tensor("idx", [128, ncols_idx], mybir.dt.int16,
                         kind="ExternalInput")
    COL = nc.dram_tensor("coloff", [128, nchunk_tot], f32, kind="ExternalInput")
    W1 = nc.dram_tensor("w1", [128, nchunk_tot], f32, kind="ExternalInput")
    WB = nc.dram_tensor("Wb", [4 * D, 4 * D], f32, kind="ExternalInput")
    BIASB = nc.dram_tensor("biasb", [128, 4 * D], f32, kind="ExternalInput")
    OUT = nc.dram_tensor("OUT", [NPAD, D], f32, kind="ExternalOutput")

    cc_in = [nc.dram_tensor(f"cc_in{q}", [QSZ[q], D], f32)
             for q in range(NSC)]
    cc_out = [nc.dram_tensor(f"cc_out{q}", [NCORES * QSZ[q], D], f32,
                             addr_space="Shared") for q in range(NSC)]

    qn = [0]

    def next_q():
        q = qn[0] % 4
        qn[0] += 1
        return q

    with tile.TileContext(nc) as tc:
        import contextlib
        ctx = contextlib.ExitStack()
        with ctx:
            const = ctx.enter_context(tc.tile_pool(name="const", bufs=1))
            gp = ctx.enter_context(tc.tile_pool(name="g", bufs=14))
            sp = ctx.enter_context(tc.tile_pool(name="selw", bufs=12))
            ldp = ctx.enter_context(tc.tile_pool(name="ld", bufs=12))
            tp = ctx.enter_context(tc.tile_pool(name="tt", bufs=12))
            outp = ctx.enter_context(tc.tile_pool(name="outp", bufs=6))
            ps_a = ctx.enter_context(tc.tile_pool(name="ps_a", bufs=4, space="PSUM"))
            ps_t = ctx.enter_context(tc.tile_pool(name="ps_t", bufs=2, space="PSUM"))
            ps_g = ctx.enter_context(tc.tile_pool(name="ps_g", bufs=2, space="PSUM"))

            # --- resident constants -----------------------------------------
            idx_sb = const.tile([128, ncols_idx], mybir.dt.int16)
            nc.sync.dma_start(out=idx_sb[:], in_=IDX[:])
            col_sb = const.tile([128, nchunk_tot], f32)
            nc.sync.dma_start(out=col_sb[:], in_=COL[:])
            w1_sb = const.tile([128, nchunk_tot], f32)
            nc.sync.dma_start(out=w1_sb[:], in_=W1[:])
            wb_sb = [const.tile([128, 4 * D], f32, tag=f"wb{i}", name=f"wb{i}")
                     for i in range(4)]
            for i in range(4):
                nc.sync.dma_start(out=wb_sb[i][:], in_=WB[i * 128:(i + 1) * 128, :])
            biasb_sb = const.tile([128, 4 * D], f32)
            nc.sync.dma_start(out=biasb_sb[:], in_=BIASB[:])
            ident = const.tile([128, 128], f32)
            make_identity(nc, ident[:])
            iota_i = const.tile([128, 128], mybir.dt.int32)
            nc.gpsimd.iota(iota_i[:], pattern=[[1, 128]], base=0,
                           channel_multiplier=0)
            iota_f = const.tile([128, 128], f32)
            nc.vector.tensor_copy(out=iota_f[:], in_=iota_i[:])

            idx_cols_per_tile = ntot // 16

            def scatter_tile(t, src_tensors, w_sb, transposed):
                """Accumulate one col tile's scatter into a PSUM tile."""
                ps = ps_a.tile([128, 128], f32, tag="scat")
                ch = 0
                for s in range(NSC):
                    g = gp.tile([128, capmax, 128], f32, tag="g")
                    icol0 = t * idx_cols_per_tile + int(
                        sum(cap[:s])) * 8
                    nc.gpsimd.dma_gather(
                        out_ap=g[:, :cap[s], :],
                        in_ap=src_tensors[s][:],
                        idxs_ap=idx_sb[:, icol0:icol0 + cap[s] * 8],
                        num_idxs=cap[s] * 128,
                        num_idxs_reg=cap[s] * 128,
                        elem_size=D,
                        queue_num=next_q(),
                    )
                    for k in range(cap[s]):
                        j = t * nchunks + ch
                        selw = sp.tile([128, 128], f32, tag="selw")
                        nc.vector.tensor_scalar(
                            out=selw[:],
                            in0=iota_f[:],
                            scalar1=col_sb[:, j:j + 1],
                            scalar2=w_sb[:, j:j + 1],
                            op0=mybir.AluOpType.is_equal,
                            op1=mybir.AluOpType.mult,
                        )
                        if transposed:
                            nc.tensor.matmul(ps[:], lhsT=g[:, k, :], rhs=selw[:],
                                             start=(ch == 0),
                                             stop=(ch == nchunks - 1))
                        else:
                            nc.tensor.matmul(ps[:], lhsT=selw[:], rhs=g[:, k, :],
                                             start=(ch == 0),
                                             stop=(ch == nchunks - 1))
                        ch += 1
                return ps

            # --- phase A: prop1 --------------------------------------------
            def phase_a(iv=None):
                for t in range(TILES):
                    ps = scatter_tile(t, Hc, w1_sb, transposed=False)
                    tx1 = outp.tile([128, 128], f32, tag="tx1", name="tx1")
                    nc.vector.tensor_copy(out=tx1[:], in_=ps[:])
                    rows = min(128, NPC - t * 128)
                    q = min((t * 128) // 3968, 3)
                    off = t * 128 - QB[q]
                    nc.sync.dma_start(out=cc_in[q][off:off + rows, :],
                                      in_=tx1[:rows, :])

            def phase_b():
                for q in range(NSC):
                    nc.gpsimd.collective_compute(
                        "AllGather",
                        mybir.AluOpType.bypass,
                        replica_groups=[list(range(NCORES))],
                        ins=[cc_in[q][:]],
                        outs=[cc_out[q][:]],
                    )

            Tc = [cc_out[s][:] for s in range(NSC)]

            def transpose_to(sb_tile, src_tile):
                pst = ps_t.tile([128, 128], f32, tag="tr")
                nc.tensor.transpose(out=pst[:], in_=src_tile[:], identity=ident[:])
                nc.vector.tensor_copy(out=sb_tile[:], in_=pst[:])

            # --- phase C: prop2 + dense + LSTM -----------------------------
            def phase_c(iv=None):
              for t in range(TILES):
                  ps2 = scatter_tile(t, Tc, w1_sb, transposed=True)  # [f x n] L@Tx1

                  ct = ldp.tile([128, 128], f32, tag="ct")
                  nc.sync.dma_start(out=ct[:], in_=Cp[t * 128:(t + 1) * 128, :])
                  # reload own Tx1 tile from the device-local collective input
                  rows = min(128, NPC - t * 128)
                  q = min((t * 128) // 3968, 3)
                  off = t * 128 - QB[q]
                  t1t = ldp.tile([128, 128], f32, tag="t1t")
                  nc.sync.dma_start(out=t1t[:rows, :],
                                    in_=cc_in[q][off:off + rows, :])

                  xT = tp.tile([128, 128], f32, tag="xT")
                  nc.sync.dma_start(out=xT[:], in_=XTd[:, t * 128:(t + 1) * 128])
                  hT = tp.tile([128, 128], f32, tag="hT")
                  nc.sync.dma_start(out=hT[:], in_=HTd[:, t * 128:(t + 1) * 128])
                  t1T = tp.tile([128, 128], f32, tag="t1T")
                  transpose_to(t1T, t1t)
                  t2T = tp.tile([128, 128], f32, tag="t2T")
                  nc.vector.scalar_tensor_tensor(
                      out=t2T[:], in0=ps2[:], scalar=2.0, in1=hT[:],
                      op0=mybir.AluOpType.mult, op1=mybir.AluOpType.subtract)

                  gps = ps_g.tile([128, 4 * D], f32, tag="G")
                  nc.tensor.matmul(gps[:], lhsT=xT[:], rhs=wb_sb[0][:],
                                   start=True, stop=False)
                  nc.tensor.matmul(gps[:], lhsT=hT[:], rhs=wb_sb[1][:],
                                   start=False, stop=False)
                  nc.tensor.matmul(gps[:], lhsT=t1T[:], rhs=wb_sb[2][:],
                                   start=False, stop=False)
                  nc.tensor.matmul(gps[:], lhsT=t2T[:], rhs=wb_sb[3][:],
                                   start=False, stop=True)

                  gs = outp.tile([128, 4 * D], f32, tag="gs")
                  nc.vector.tensor_tensor(out=gs[:], in0=gps[:], in1=biasb_sb[:],
                                          op=mybir.AluOpType.add)
                  act = outp.tile([128, 4 * D], f32, tag="act")
                  AF = mybir.ActivationFunctionType
                  nc.scalar.activation(out=act[:, 0:128], in_=gs[:, 0:128],
                                       func=AF.Sigmoid)
                  nc.scalar.activation(out=act[:, 128:256], in_=gs[:, 128:256],
                                       func=AF.Sigmoid)
                  nc.scalar.activation(out=act[:, 256:384], in_=gs[:, 256:384],
                                       func=AF.Tanh)
                  nc.scalar.activation(out=act[:, 384:512], in_=gs[:, 384:512],
                                       func=AF.Sigmoid)

                  fc = outp.tile([128, 128], f32, tag="fc")
                  nc.vector.tensor_tensor(out=fc[:], in0=act[:, 128:256], in1=ct[:],
                                          op=mybir.AluOpType.mult)
                  it = outp.tile([128, 128], f32, tag="it")
                  nc.vector.tensor_tensor(out=it[:], in0=act[:, 0:128],
                                          in1=act[:, 256:384],
                                          op=mybir.AluOpType.mult)
                  cn = outp.tile([128, 128], f32, tag="cn")
                  nc.vector.tensor_tensor(out=cn[:], in0=fc[:], in1=it[:],
                                          op=mybir.AluOpType.add)
                  tc_t = outp.tile([128, 128], f32, tag="tc")
                  nc.scalar.activation(out=tc_t[:], in_=cn[:], func=AF.Tanh)
                  hn = outp.tile([128, 128], f32, tag="hn")
                  nc.vector.tensor_tensor(out=hn[:], in0=act[:, 384:512],
                                          in1=tc_t[:], op=mybir.AluOpType.mult)
                  nc.sync.dma_start(out=OUT[t * 128:(t + 1) * 128, :], in_=hn[:])


            if variant == "full":
                for _rep in range(reps):
                    phase_a()
                    phase_b()
                    phase_c()
            elif variant == "a_only":
                tc.For_i_unrolled(0, reps, 1, phase_a, max_unroll=1)
            elif variant == "c_only":
                tc.For_i_unrolled(0, reps, 1, phase_c, max_unroll=1)
            else:
                raise ValueError(variant)

    nc.compile()
    return nc


def kernel(X, edge_index, edge_weight, H, C, W, b, conv_W, conv_b):
    from concourse.bass_utils import run_bass_kernel_spmd

    in_maps, meta = _host_prep(X, edge_index, edge_weight, H, C, W, b,
                               conv_W, conv_b)
    key = (meta["cap"],)
    if key not in _CACHE:
        _CACHE[key] = _build_program(meta)
    nc = _CACHE[key]

    res = run_bass_kernel_spmd(nc, in_maps, list(range(NCORES)))
    out = np.empty((N, D), np.float32)
    for d in range(NCORES):
        out[d * NPC:(d + 1) * NPC] = res.results[d]["OUT"][:NPC]
    return out



# revision 2
# speedup vs baseline: 1.1411x; 1.1411x over previous
"""GCLSTM cell (Chebyshev K=3 GCN-gated LSTM) on 8 Trainium2 NeuronCores, v2.

Sharding: nodes partitioned contiguously across 8 cores (12500 each); each
core owns its node rows and incoming edges (partitioned by destination).
Host pre-normalizes edge weights and sorts/pads per-device edges by
(destination tile, source quarter-block).

v2 changes vs baseline:
  - bf16 gather sources / scatter matmuls (FWL) / dense matmuls / AllGather
  - trailing -1 gather indices skip device-uniform pad tails (fewer
    descriptors + bytes); first POOL_WARM tiles keep full gathers so pool
    buffers never expose uninitialized SBUF to the matmuls
  - AllGather for quarter q issued inline right after its last tile so the
    collective overlaps the rest of phase A
  - selw one-hot weight matrices built 1 tile at a time with 2 broadcast
    DVE ops instead of nchunks tensor_scalar ops
  - gate order (i,f,o,c) so sigmoid/tanh activations batch into 2 ACT ops
  - Tx1^T kept SBUF-resident from phase A (no reload / re-transpose in C)

diag term of L_hat is exactly 0 (lambda_max = 2), so prop is pure scatter.
"""
import numpy as np
import ml_dtypes

BF16 = ml_dtypes.bfloat16

N = 100000
D = 128
NCORES = 8
NPC = N // NCORES            # 12500 nodes per core
TILES = (NPC + 127) // 128   # 98
NPAD = TILES * 128           # 12544
# Quarter-block source chunking: gather-source block is the concat over
# ranks of one quarter of each rank's shard (block <= 31744 rows, under the
# int16 dma_gather index limit). Quarter boundaries are tile(128)-aligned.
QB = [0, 3968, 7936, 11904, 12500]
QSZ = [QB[i + 1] - QB[i] for i in range(4)]      # 3968,3968,3968,596
NSC = 4
POOL_WARM = 14               # tiles whose gathers fill pad lanes (no -1 skip)
GP_BUFS = 12

_CACHE = {}


def _host_prep(X, edge_index, edge_weight, H, C, W, b, conv_W, conv_b):
    row = np.asarray(edge_index[0], dtype=np.int64)
    col = np.asarray(edge_index[1], dtype=np.int64)
    ew = np.asarray(edge_weight, dtype=np.float32)

    deg = np.bincount(row, weights=ew.astype(np.float64), minlength=N)
    deg = deg.astype(np.float32)
    dinv = np.where(deg > 0, deg ** -0.5, 0.0).astype(np.float32)
    w = -(dinv[row] * ew * dinv[col])  # 2/lambda_max == 1

    dev = col // NPC
    counts = np.zeros((NCORES, TILES, NSC), dtype=np.int64)
    lsrc = row % NPC
    q_of = np.minimum(lsrc // 3968, 3)
    order = np.argsort(dev * (TILES * NSC) +
                       ((col % NPC) // 128) * NSC + q_of,
                       kind="stable")
    row_s, col_s, w_s, dev_s = row[order], col[order], w[order], dev[order]
    colloc_s = col_s % NPC
    tile_s = colloc_s // 128
    lsrc_s = row_s % NPC
    dsrc_s = row_s // NPC
    sc_s = np.minimum(lsrc_s // 3968, 3)
    qb = np.array(QB[:4], dtype=np.int64)
    qsz = np.array(QSZ, dtype=np.int64)
    blockrow_s = dsrc_s * qsz[sc_s] + (lsrc_s - qb[sc_s])
    np.add.at(counts, (dev_s, tile_s, sc_s), 1)

    # Uniform chunk capacity per src chunk (same program on all cores)
    cap = np.zeros(NSC, dtype=np.int64)
    for s in range(NSC):
        cap[s] = int(np.ceil(counts[:, :, s].max() / 128))
    cap = np.maximum(cap, 1)
    ntot = int(cap.sum()) * 128          # padded edges per tile
    nchunks = int(cap.sum())             # 128-edge chunks per tile

    # valid gather length per (tile, q): max over devices, device-uniform so
    # num_idxs_reg can be a compile-time constant. Tiles < POOL_WARM gather
    # their full padded range to flush uninitialized SBUF from pool buffers.
    count_pad = counts.max(axis=0)                     # [TILES, NSC]
    count_pad = np.maximum(count_pad, 1)
    for t in range(min(POOL_WARM, TILES)):
        count_pad[t] = cap * 128
    count_pad = np.minimum(count_pad, cap[None, :] * 128)

    # slot base offset of (tile, src_chunk) within a device's padded edges
    sc_base = np.concatenate([[0], np.cumsum(cap)[:-1]]) * 128

    idx16 = np.zeros((NCORES, TILES * ntot), dtype=np.int16)
    coloff = np.zeros((NCORES, TILES * ntot // 128), dtype=np.float32)
    wpad = np.zeros((NCORES, TILES * ntot // 128 * 128), dtype=np.float32)
    coloff = np.zeros((NCORES, TILES * ntot), dtype=np.float32)
    wpad = np.zeros((NCORES, TILES * ntot), dtype=np.float32)

    # mark device-uniform pad tails as -1 (skipped by the gather)
    for t in range(TILES):
        for s in range(NSC):
            lo = t * ntot + int(sc_base[s]) + int(count_pad[t, s])
            hi = t * ntot + int(sc_base[s]) + int(cap[s]) * 128
            if lo < hi:
                idx16[:, lo:hi] = -1

    # position of each edge within its (dev, tile, sc) bucket
    key = dev_s * (TILES * NSC) + tile_s * NSC + sc_s
    same = key[1:] == key[:-1]
    idxs = np.arange(len(key))
    starts = np.concatenate([[0], idxs[1:][~same]])
    runid = np.cumsum(np.concatenate([[0], (~same).astype(np.int64)]))
    rank = idxs - starts[runid]

    slot = tile_s * ntot + sc_base[sc_s] + rank
    idx16[dev_s, slot] = blockrow_s.astype(np.int16)
    coloff[dev_s, slot] = (colloc_s % 128).astype(np.float32)
    wpad[dev_s, slot] = w_s

    # SBUF layouts:
    #  - dma_gather idx: idx i -> partition i%16, column i//16; replicate x8
    #  - per-chunk scalars (coloff, w): lane e -> partition e, column chunk
    ncols_idx = TILES * ntot // 16
    nchunk_tot = TILES * nchunks
    per_dev = []
    for d in range(NCORES):
        a = idx16[d].reshape(-1, 16).T            # [16, ncols_idx]
        ii = np.tile(a, (8, 1))                   # replicate to 128 partitions
        co = coloff[d].reshape(nchunk_tot, 128).T.astype(BF16)
        ww = wpad[d].reshape(nchunk_tot, 128).T.astype(BF16)
        per_dev.append((ii, co, ww))

    # fused dense weights: rhs blocks [128f, 512gc] for X, H(Tx0), Tx1, Tx2
    # gate column order i, f, o, c (sigmoid gates first for batched ACT)
    perm = [0, 1, 3, 2]
    Wb = np.zeros((4, D, 4 * D), dtype=np.float32)
    for gi, g in enumerate(perm):
        Wb[0][:, gi * D:(gi + 1) * D] = W[g]
        Wb[1][:, gi * D:(gi + 1) * D] = conv_W[g, 0]
        Wb[2][:, gi * D:(gi + 1) * D] = conv_W[g, 1]
        Wb[3][:, gi * D:(gi + 1) * D] = conv_W[g, 2]
    bias = np.concatenate([b[g] + conv_b[g] for g in perm]).astype(np.float32)
    biasb = np.tile(bias[None, :], (128, 1))

    Xp = np.zeros((NCORES, NPAD, D), np.float32)
    Hp = np.zeros((NCORES, NPAD, D), np.float32)
    Cp = np.zeros((NCORES, NPAD, D), np.float32)
    Xp[:, :NPC] = np.asarray(X, np.float32).reshape(NCORES, NPC, D)
    Hp[:, :NPC] = np.asarray(H, np.float32).reshape(NCORES, NPC, D)
    Cp[:, :NPC] = np.asarray(C, np.float32).reshape(NCORES, NPC, D)
    XT = np.ascontiguousarray(np.transpose(Xp, (0, 2, 1)))  # [NC, D, NPAD]
    HT = np.ascontiguousarray(np.transpose(Hp, (0, 2, 1)))
    Cb = Cp

    Hsh = np.asarray(H, np.float32).reshape(NCORES, NPC, D)
    hc = []
    for q in range(NSC):
        hc.append(np.ascontiguousarray(
            Hsh[:, QB[q]:QB[q + 1], :].reshape(-1, D)).astype(BF16))

    in_maps = []
    for d in range(NCORES):
        ii, co, ww = per_dev[d]
        m = {
            "XT": XT[d], "HT": HT[d], "Cp": Cb[d],
            "idx": np.ascontiguousarray(ii),
            "coloff": np.ascontiguousarray(co),
            "w1": np.ascontiguousarray(ww),
            "Wb": Wb.reshape(4 * D, 4 * D),
            "biasb": biasb,
        }
        for s in range(NSC):
            m[f"Hc{s}"] = hc[s]
        in_maps.append(m)

    meta = dict(cap=tuple(int(c) for c in cap), ntot=ntot, nchunks=nchunks,
                ncols_idx=ncols_idx, nchunk_tot=nchunk_tot,
                count_pad=tuple(tuple(int(c) for c in row)
                                for row in count_pad))
    return in_maps, meta


def _build_program(meta, variant="full", reps=1):
    import concourse.bass as bass  # noqa: F401
    import concourse.bacc as bacc
    import concourse.tile as tile
    from concourse import mybir
    from concourse.masks import make_identity

    cap = meta["cap"]
    ntot = meta["ntot"]
    nchunks = meta["nchunks"]
    ncols_idx = meta["ncols_idx"]
    nchunk_tot = meta["nchunk_tot"]
    count_pad = meta["count_pad"]
    capmax = max(cap)
    f32 = mybir.dt.float32
    bf16 = mybir.dt.bfloat16

    nc = bacc.Bacc("TRN2", target_bir_lowering=False, debug=False,
                   num_devices=NCORES, num_swdge_queues=4)

    Hc = [nc.dram_tensor(f"Hc{s}", [NCORES * QSZ[s], D], bf16,
                         kind="ExternalInput") for s in range(NSC)]
    XTd = nc.dram_tensor("XT", [D, NPAD], f32, kind="ExternalInput")
    HTd = nc.dram_tensor("HT", [D, NPAD], f32, kind="ExternalInput")
    Cp = nc.dram_tensor("Cp", [NPAD, D], f32, kind="ExternalInput")
    IDX = nc.dram_tensor("idx", [128, ncols_idx], mybir.dt.int16,
                         kind="ExternalInput")
    COL = nc.dram_tensor("coloff", [128, nchunk_tot], bf16,
                         kind="ExternalInput")
    W1 = nc.dram_tensor("w1", [128, nchunk_tot], bf16, kind="ExternalInput")
    WB = nc.dram_tensor("Wb", [4 * D, 4 * D], f32, kind="ExternalInput")
    BIASB = nc.dram_tensor("biasb", [128, 4 * D], f32, kind="ExternalInput")
    OUT = nc.dram_tensor("OUT", [NPAD, D], f32, kind="ExternalOutput")

    cc_in = [nc.dram_tensor(f"cc_in{q}", [QSZ[q], D], bf16)
             for q in range(NSC)]
    cc_out = [nc.dram_tensor(f"cc_out{q}", [NCORES * QSZ[q], D], bf16,
                             addr_space="Shared") for q in range(NSC)]

    qn = [0]

    def next_q():
        q = qn[0] % 4
        qn[0] += 1
        return q

    # tile t -> quarter q and row range
    def tile_q(t):
        return min((t * 128) // 3968, 3)

    qlast = {}
    for t in range(TILES):
        qlast[tile_q(t)] = t

    with tile.TileContext(nc) as tc:
        import contextlib
        ctx = contextlib.ExitStack()
        with ctx:
            const = ctx.enter_context(tc.tile_pool(name="const", bufs=1))
            gp = ctx.enter_context(tc.tile_pool(name="g", bufs=GP_BUFS))
            sp = ctx.enter_context(tc.tile_pool(name="selw", bufs=4))
            ldp = ctx.enter_context(tc.tile_pool(name="ld", bufs=8))
            tp = ctx.enter_context(tc.tile_pool(name="tt", bufs=10))
            outp = ctx.enter_context(tc.tile_pool(name="outp", bufs=8))
            ps_a = ctx.enter_context(
                tc.tile_pool(name="ps_a", bufs=4, space="PSUM"))
            ps_t = ctx.enter_context(
                tc.tile_pool(name="ps_t", bufs=2, space="PSUM"))
            ps_g = ctx.enter_context(
                tc.tile_pool(name="ps_g", bufs=2, space="PSUM"))

            # --- resident constants -----------------------------------------
            idx_sb = const.tile([128, ncols_idx], mybir.dt.int16)
            nc.sync.dma_start(out=idx_sb[:], in_=IDX[:])
            col_sb = const.tile([128, nchunk_tot], bf16)
            nc.sync.dma_start(out=col_sb[:], in_=COL[:])
            w1_sb = const.tile([128, nchunk_tot], bf16)
            nc.sync.dma_start(out=w1_sb[:], in_=W1[:])
            # X/H dense matmuls in fp32 (precision), Tx1/Tx2 in bf16
            wb_sb = [const.tile([128, 4 * D], f32, tag=f"wb{i}",
                                name=f"wb{i}") for i in range(2)]
            for i in range(2):
                nc.sync.dma_start(out=wb_sb[i][:],
                                  in_=WB[i * 128:(i + 1) * 128, :])
            wb_bf = [const.tile([128, 4 * D], bf16, tag=f"wbb{i}",
                                name=f"wbb{i}") for i in range(2)]
            for i in range(2):
                wtmp = const.tile([128, 4 * D], f32, tag=f"wtmp{i}")
                nc.sync.dma_start(out=wtmp[:],
                                  in_=WB[(2 + i) * 128:(3 + i) * 128, :])
                nc.vector.tensor_copy(out=wb_bf[i][:], in_=wtmp[:])
            biasb_sb = const.tile([128, 4 * D], f32)
            nc.sync.dma_start(out=biasb_sb[:], in_=BIASB[:])
            ident = const.tile([128, 128], f32)
            make_identity(nc, ident[:])
            ident_bf = const.tile([128, 128], bf16)
            nc.vector.tensor_copy(out=ident_bf[:], in_=ident[:])
            iota_i = const.tile([128, 128], mybir.dt.int32)
            nc.gpsimd.iota(iota_i[:], pattern=[[1, 128]], base=0,
                           channel_multiplier=0)
            iota_bf = const.tile([128, 128], bf16)
            nc.vector.tensor_copy(out=iota_bf[:], in_=iota_i[:])
            # iota repeated per chunk: [128, nchunks, 128] with value c
            iota_rep = const.tile([128, nchunks, 128], bf16)
            for ch in range(nchunks):
                nc.vector.tensor_copy(out=iota_rep[:, ch, :], in_=iota_bf[:])
            # Tx1^T resident across phases: [128f, TILES*128]
            t1T_all = const.tile([128, TILES * 128], bf16, name="t1T_all")

            idx_cols_per_tile = ntot // 16

            g_const = [None]

            def scatter_tile(t, src_tensors, transposed,
                             do_gather=True, do_compute=True):
                """Accumulate one col tile's scatter into a PSUM tile."""
                gtiles = []
                for s in range(NSC):
                    if do_compute and not do_gather:
                        gtiles.append(g_const[0])
                        continue
                    g = gp.tile([128, capmax, 128], bf16, tag="g")
                    icol0 = t * idx_cols_per_tile + int(
                        sum(cap[:s])) * 8
                    nv = int(count_pad[t][s])
                    nc.gpsimd.dma_gather(
                        out_ap=g[:, :cap[s], :],
                        in_ap=src_tensors[s][:],
                        idxs_ap=idx_sb[:, icol0:icol0 + cap[s] * 8],
                        num_idxs=cap[s] * 128,
                        num_idxs_reg=nv,
                        elem_size=D,
                        queue_num=next_q(),
                    )
                    gtiles.append(g)
                if not do_compute:
                    return None
                ps = ps_a.tile([128, 128], f32, tag="scat")
                selw = sp.tile([128, nchunks, 128], bf16, tag="selw")
                c0 = t * nchunks
                nc.vector.tensor_tensor(
                    out=selw[:],
                    in0=iota_rep[:],
                    in1=col_sb[:, c0:c0 + nchunks].to_broadcast(
                        [128, nchunks, 128]),
                    op=mybir.AluOpType.is_equal,
                )
                nc.vector.tensor_tensor(
                    out=selw[:],
                    in0=selw[:],
                    in1=w1_sb[:, c0:c0 + nchunks].to_broadcast(
                        [128, nchunks, 128]),
                    op=mybir.AluOpType.mult,
                )
                ch = 0
                for s in range(NSC):
                    for k in range(cap[s]):
                        if transposed:
                            nc.tensor.matmul(ps[:], lhsT=gtiles[s][:, k, :],
                                             rhs=selw[:, ch, :],
                                             start=(ch == 0),
                                             stop=(ch == nchunks - 1))
                        else:
                            nc.tensor.matmul(ps[:], lhsT=selw[:, ch, :],
                                             rhs=gtiles[s][:, k, :],
                                             start=(ch == 0),
                                             stop=(ch == nchunks - 1))
                        ch += 1
                return ps

            def issue_allgather(q):
                nc.gpsimd.collective_compute(
                    "AllGather",
                    mybir.AluOpType.bypass,
                    replica_groups=[list(range(NCORES))],
                    ins=[cc_in[q][:]],
                    outs=[cc_out[q][:]],
                )

            # --- phase A: prop1 (+ inline AllGather per quarter) -----------
            def phase_a(iv=None, collectives=True,
                        do_gather=True, do_compute=True):
                for t in range(TILES):
                    ps = scatter_tile(t, Hc, transposed=False,
                                      do_gather=do_gather,
                                      do_compute=do_compute)
                    if ps is None:
                        continue
                    tx1 = outp.tile([128, 128], bf16, tag="tx1", name="tx1")
                    nc.vector.tensor_copy(out=tx1[:], in_=ps[:])
                    rows = min(128, NPC - t * 128)
                    q = tile_q(t)
                    off = t * 128 - QB[q]
                    nc.sync.dma_start(out=cc_in[q][off:off + rows, :],
                                      in_=tx1[:rows, :])
                    # transpose Tx1 tile while hot; keep resident for phase C
                    pst = ps_t.tile([128, 128], bf16, tag="tr")
                    nc.tensor.transpose(out=pst[:], in_=tx1[:],
                                        identity=ident_bf[:])
                    nc.vector.tensor_copy(
                        out=t1T_all[:, t * 128:(t + 1) * 128], in_=pst[:])
                    if collectives and qlast[q] == t:
                        issue_allgather(q)

            Tc = [cc_out[s][:] for s in range(NSC)]

            # --- phase C: prop2 + dense + LSTM -----------------------------
            def phase_c(iv=None):
                AF = mybir.ActivationFunctionType
                for t in range(TILES):
                    ps2 = scatter_tile(t, Tc, transposed=True)  # [f,n] L@Tx1

                    ct = ldp.tile([128, 128], f32, tag="ct")
                    nc.sync.dma_start(out=ct[:],
                                      in_=Cp[t * 128:(t + 1) * 128, :])
                    xT = tp.tile([128, 128], f32, tag="xT")
                    nc.sync.dma_start(out=xT[:],
                                      in_=XTd[:, t * 128:(t + 1) * 128])
                    hT = tp.tile([128, 128], f32, tag="hT")
                    nc.sync.dma_start(out=hT[:],
                                      in_=HTd[:, t * 128:(t + 1) * 128])
                    t2T = tp.tile([128, 128], bf16, tag="t2T")
                    nc.vector.scalar_tensor_tensor(
                        out=t2T[:], in0=ps2[:], scalar=2.0, in1=hT[:],
                        op0=mybir.AluOpType.mult,
                        op1=mybir.AluOpType.subtract)

                    gps = ps_g.tile([128, 4 * D], f32, tag="G")
                    nc.tensor.matmul(gps[:], lhsT=xT[:], rhs=wb_sb[0][:],
                                     start=True, stop=False)
                    nc.tensor.matmul(gps[:], lhsT=hT[:], rhs=wb_sb[1][:],
                                     start=False, stop=False)
                    nc.tensor.matmul(gps[:],
                                     lhsT=t1T_all[:, t * 128:(t + 1) * 128],
                                     rhs=wb_bf[0][:],
                                     start=False, stop=False)
                    nc.tensor.matmul(gps[:], lhsT=t2T[:], rhs=wb_bf[1][:],
                                     start=False, stop=True)

                    gs = outp.tile([128, 4 * D], f32, tag="gs")
                    nc.vector.tensor_tensor(out=gs[:], in0=gps[:],
                                            in1=biasb_sb[:],
                                            op=mybir.AluOpType.add)
                    act = outp.tile([128, 4 * D], f32, tag="act")
                    # gates i,f,o are sigmoid (cols 0:384), c is tanh
                    nc.scalar.activation(out=act[:, 0:384], in_=gs[:, 0:384],
                                         func=AF.Sigmoid)
                    nc.scalar.activation(out=act[:, 384:512],
                                         in_=gs[:, 384:512], func=AF.Tanh)

                    fc = outp.tile([128, 128], f32, tag="fc")
                    nc.vector.tensor_tensor(out=fc[:], in0=act[:, 128:256],
                                            in1=ct[:],
                                            op=mybir.AluOpType.mult)
                    it = outp.tile([128, 128], f32, tag="it")
                    nc.vector.tensor_tensor(out=it[:], in0=act[:, 0:128],
                                            in1=act[:, 384:512],
                                            op=mybir.AluOpType.mult)
                    cn = outp.tile([128, 128], f32, tag="cn")
                    nc.vector.tensor_tensor(out=cn[:], in0=fc[:], in1=it[:],
                                            op=mybir.AluOpType.add)
                    tc_t = outp.tile([128, 128], f32, tag="tc")
                    nc.scalar.activation(out=tc_t[:], in_=cn[:], func=AF.Tanh)
                    hn = outp.tile([128, 128], f32, tag="hn")
                    nc.vector.tensor_tensor(out=hn[:], in0=act[:, 256:384],
                                            in1=tc_t[:],
                                            op=mybir.AluOpType.mult)
                    nc.sync.dma_start(out=OUT[t * 128:(t + 1) * 128, :],
                                      in_=hn[:])

            if variant == "full":
                for _rep in range(reps):
                    phase_a()
                    phase_c()
            elif variant == "a_only":
                tc.For_i_unrolled(
                    0, reps, 1,
                    lambda iv: phase_a(iv, collectives=False),
                    max_unroll=1)
            elif variant == "a_gather":
                # gathers only — isolates DMA/Pool cost
                tc.For_i_unrolled(
                    0, reps, 1,
                    lambda iv: phase_a(iv, collectives=False,
                                       do_compute=False),
                    max_unroll=1)
            elif variant == "a_compute":
                # selw + matmuls on a pre-gathered const tile — isolates
                # DVE/TensorE cost
                gc_t = const.tile([128, capmax, 128], bf16, name="g_const")
                nc.gpsimd.dma_gather(
                    out_ap=gc_t[:, :, :], in_ap=Hc[0][:],
                    idxs_ap=idx_sb[:, 0:capmax * 8],
                    num_idxs=capmax * 128, num_idxs_reg=capmax * 128,
                    elem_size=D, queue_num=0)
                g_const[0] = gc_t
                tc.For_i_unrolled(
                    0, reps, 1,
                    lambda iv: phase_a(iv, collectives=False,
                                       do_gather=False),
                    max_unroll=1)
            elif variant == "c_only":
                for tt in range(TILES):
                    nc.vector.tensor_copy(
                        out=t1T_all[:, tt * 128:(tt + 1) * 128],
                        in_=iota_bf[:])
                tc.For_i_unrolled(0, reps, 1, phase_c, max_unroll=1)
            else:
                raise ValueError(variant)

    nc.compile()
    return nc


def kernel(X, edge_index, edge_weight, H, C, W, b, conv_W, conv_b):
    from concourse.bass_utils import run_bass_kernel_spmd

    in_maps, meta = _host_prep(X, edge_index, edge_weight, H, C, W, b,
                               conv_W, conv_b)
    key = (meta["cap"], meta["count_pad"])
    if key not in _CACHE:
        _CACHE[key] = _build_program(meta)
    nc = _CACHE[key]

    res = run_bass_kernel_spmd(nc, in_maps, list(range(NCORES)))
    out = np.empty((N, D), np.float32)
    for d in range(NCORES):
        out[d * NPC:(d + 1) * NPC] = res.results[d]["OUT"][:NPC]
    return out
